# revision 1
# baseline (speedup 1.0000x reference)
"""Trainium2 Bass kernel for nn_CapsuleLayer_9852654977072.

The reference module collapses mathematically: the routing loop's coupling
logits `b` stay zero (faithfully-reproduced bug in the original torch code),
so routing coefficients are a fixed spatial map r(h,w) = 1/(8*cnt(h,w)) where
cnt is the 5x5 box-count inside the image. The whole module is therefore:

    p = conv2d(u as [N,64,H,W], Wd as [128,64,5,5], pad=2) * s(h,w)
    v = squash_z1(p)   # groups of 16 channels
    out[n,t1,z1,h,w] = v

Device strategy (8 cores, SPMD): shard (batch n in 0..3) x (row-half in 0..1).
Each core computes all 128 output channels for 64 rows of one image.

Conv: inputs shipped as XA/XC [128, 68, 132] whose partition halves hold u
shifted by (+0row,+1row) and (+2row+0col,+2row+1col) respectively, columns
padded by 2. Per 4-row block, 13 PSUM-accumulated fp32r matmuls (N=512, full
PE rate) cover all 25 taps: 10 XA row-pairs + 2 XC col-pairs + 1 K=64 single.

Squash: square (ACT) -> block-diag matmul (n2 over z1) -> factor on
8-partition tiles with the spatial scale folded in via a s^2 map
(F = y/((1+y)sqrt(y_raw+eps)), y = s^2*y_raw) -> expand matmul -> v = p*F.
"""

import numpy as np

T0, Z0, T1, Z1, KK, PAD = 4, 16, 8, 16, 5, 2
N, H, W_SP = 4, 128, 128
CIN, COUT = T0 * Z0, T1 * Z1  # 64, 128
N_CORES = 8
ROWS = 64          # output rows per core
XROWS = 68         # input rows incl. halo
XCOLS = 132        # 128 + 2*PAD
BLK = 4            # output rows per block
N_BLKS = ROWS // BLK

# conv matmul j -> (source, row_off, col_off); weights match in _weight_tiles
_MM_SLICES = (
    [('XA', dy + 2, dx + 2) for dy in (-2, 0) for dx in (-2, -1, 0, 1, 2)]
    + [('XC', 2, 0), ('XC', 2, 2), ('XC', 2, 4)]
)

_CACHE = {}


def _weight_tiles(W):
    Wd = W.transpose(1, 0, 2, 3, 4).reshape(COUT, CIN, KK, KK)
    wl = np.zeros((128, 13, 128), np.float32)  # [k, j, m]
    j = 0
    for dy in (-2, 0):
        for dx in (-2, -1, 0, 1, 2):
            wl[0:64, j, :] = Wd[:, :, dy + 2, dx + 2].T
            wl[64:128, j, :] = Wd[:, :, dy + 3, dx + 2].T
            j += 1
    for dx0 in (-2, 0):
        wl[0:64, j, :] = Wd[:, :, 4, dx0 + 2].T
        wl[64:128, j, :] = Wd[:, :, 4, dx0 + 3].T
        j += 1
    wl[0:64, j, :] = Wd[:, :, 4, 4].T  # single tap (2,2) on lo partitions
    return wl


def _inputs_core(x, half):
    """x: [64, H, W] one image channel-major. Returns XA, XC [128, 68, 132]."""
    base = half * 64 - 2
    XA = np.zeros((128, XROWS, XCOLS), np.float32)
    XC = np.zeros((128, XROWS, XCOLS), np.float32)

    def fill(dst, roff, c0, c1):
        lo, hi = max(0, -(base + roff)), min(XROWS, H - base - roff)
        dst[:, lo:hi, c0:c1] = x[:, base + roff + lo:base + roff + hi, :]

    fill(XA[0:64], 0, 2, 130)
    fill(XA[64:128], 1, 2, 130)
    fill(XC[0:64], 2, 2, 130)
    fill(XC[64:128], 2, 1, 129)
    return XA, XC


def _s2_map(half):
    idx = np.arange(H)
    cnt = (np.minimum(idx + 2, H - 1) - np.maximum(idx - 2, 0) + 1).astype(np.float64)
    s = 1.0 / (8.0 * cnt[:, None] * cnt[None, :])  # [H, W]
    s = s[half * 64:(half + 1) * 64, :]
    return np.ascontiguousarray((s * s).astype(np.float32).reshape(1, ROWS * 128))


def _block_diag():
    bd = np.zeros((128, 8), np.float32)
    bd[np.arange(128), np.arange(128) // 16] = 1.0
    return bd


def build_nc(reps=1):
    import concourse.bass as bass
    import concourse.bacc as bacc
    import concourse.mybir as mybir
    import concourse.tile as tile

    f32 = mybir.dt.float32
    f32r = mybir.dt.float32r
    AF = mybir.ActivationFunctionType

    nc = bacc.Bacc(None, target_bir_lowering=False)
    xa_d = nc.dram_tensor("xa", [128, XROWS * XCOLS], f32r, kind="ExternalInput")
    xc_d = nc.dram_tensor("xc", [128, XROWS * XCOLS], f32r, kind="ExternalInput")
    wl_d = nc.dram_tensor("wl", [128, 13 * 128], f32r, kind="ExternalInput")
    bd_d = nc.dram_tensor("bd", [128, 8], f32r, kind="ExternalInput")
    ex_d = nc.dram_tensor("ex", [8, 128], f32r, kind="ExternalInput")
    s2_d = nc.dram_tensor("s2", [1, ROWS * 128], f32, kind="ExternalInput")
    out_d = nc.dram_tensor("out", [128, ROWS * 128], f32, kind="ExternalOutput")

    with tile.TileContext(nc) as tc:
        with (
            tc.tile_pool(name="consts", bufs=1) as consts,
            tc.tile_pool(name="work", bufs=4) as work,
            tc.tile_pool(name="small", bufs=6) as small,
            tc.tile_pool(name="pp", bufs=3, space="PSUM") as pp,
            tc.tile_pool(name="pf", bufs=2, space="PSUM") as pf,
            tc.tile_pool(name="py", bufs=2, space="PSUM") as py,
        ):
            wl = consts.tile([128, 13, 128], f32r)
            nc.sync.dma_start(
                out=wl, in_=wl_d.ap().rearrange("p (j m) -> p j m", m=128))
            bd = consts.tile([128, 8], f32r)
            nc.sync.dma_start(out=bd, in_=bd_d.ap())
            ex = consts.tile([8, 128], f32r)
            nc.sync.dma_start(out=ex, in_=ex_d.ap())
            s2_sb = consts.tile([8, ROWS, 128], f32)
            s2_ap = s2_d.ap()
            nc.sync.dma_start(
                out=s2_sb,
                in_=bass.AP(tensor=s2_ap.tensor, offset=s2_ap.offset,
                            ap=[[0, 8], [128, ROWS], [1, 128]]))
            eps_t = consts.tile([8, 1], f32)
            nc.vector.memset(eps_t[:], 1e-9)

            xa = consts.tile([128, XROWS, XCOLS], f32r)
            xc = consts.tile([128, XROWS, XCOLS], f32r)
            xa_src = xa_d.ap().rearrange("p (r c) -> p r c", c=XCOLS)
            xc_src = xc_d.ap().rearrange("p (r c) -> p r c", c=XCOLS)
            for c0 in range(0, XROWS, 17):
                nc.sync.dma_start(
                    out=xa[:, c0:c0 + 17, :], in_=xa_src[:, c0:c0 + 17, :])
                nc.sync.dma_start(
                    out=xc[:, c0:c0 + 17, :], in_=xc_src[:, c0:c0 + 17, :])

            out_v = out_d.ap().rearrange("p (r c) -> p r c", c=128)

            import contextlib
            loop_ctx = (tc.For_i(0, reps, 1,
                                 hint_engines=(mybir.EngineType.PE,
                                               mybir.EngineType.DVE,
                                               mybir.EngineType.Activation,
                                               mybir.EngineType.Pool,
                                               mybir.EngineType.SP))
                        if reps > 1 else contextlib.nullcontext())
            def stage0(blk):
                r0 = blk * BLK
                p_ps = pp.tile([128, BLK, 128], f32)
                for j, (src, roff, coff) in enumerate(_MM_SLICES):
                    xsrc = xa if src == 'XA' else xc
                    if j == 12:  # K=64 single on lo partitions
                        lhsT = wl[0:64, j, :]
                        rhs = xsrc[0:64, r0 + roff:r0 + roff + BLK,
                                   coff:coff + 128]
                    else:
                        lhsT = wl[:, j, :]
                        rhs = xsrc[:, r0 + roff:r0 + roff + BLK, coff:coff + 128]
                    nc.tensor.matmul(p_ps[:], lhsT, rhs,
                                     start=(j == 0), stop=(j == 12))
                psq = work.tile([128, BLK, 128], f32r, tag="psq")
                nc.scalar.activation(psq[:], p_ps[:], AF.Square)
                p_sb = work.tile([128, BLK, 128], f32, tag="p_sb")
                nc.scalar.activation(p_sb[:], p_ps[:], AF.Copy, bias=0.0)
                y_ps = py.tile([8, BLK, 128], f32)
                nc.tensor.matmul(y_ps[:], bd[:], psq[:], start=True, stop=True)
                return p_sb, y_ps

            def stage1(blk, y_ps):
                r0 = blk * BLK
                # factor: F = y/((1+y)*sqrt(y_raw+eps)), y = s^2*y_raw
                a_t = small.tile([8, BLK, 128], f32, tag="a")
                nc.scalar.activation(a_t[:], y_ps[:], AF.Sqrt, bias=eps_t[:])
                y_t = small.tile([8, BLK, 128], f32, tag="y")
                nc.vector.tensor_mul(y_t[:], y_ps[:], s2_sb[:, r0:r0 + BLK, :])
                y1_t = small.tile([8, BLK, 128], f32, tag="y1")
                nc.gpsimd.tensor_scalar_add(y1_t[:], y_t[:], 1.0)
                b_t = small.tile([8, BLK, 128], f32, tag="b")
                nc.gpsimd.tensor_mul(b_t[:], a_t[:], y1_t[:])
                r_t = small.tile([8, BLK, 128], f32, tag="r")
                nc.vector.reciprocal_approx_fast(r_t[:], b_t[:])
                F_t = small.tile([8, BLK, 128], f32r, tag="F")
                nc.vector.tensor_mul(F_t[:], y_t[:], r_t[:])
                fe_ps = pf.tile([128, BLK, 128], f32)
                nc.tensor.matmul(fe_ps[:], ex[:], F_t[:], start=True, stop=True)
                return fe_ps

            def stage2(blk, p_sb, fe_ps):
                r0 = blk * BLK
                v_t = work.tile([128, BLK, 128], f32, tag="v")
                nc.vector.tensor_mul(v_t[:], p_sb[:], fe_ps[:])
                nc.sync.dma_start(out=out_v[:, r0:r0 + BLK, :], in_=v_t[:])

            with loop_ctx:
                live = {}
                for blk in range(N_BLKS + 2):
                    if blk < N_BLKS:
                        p_sb, y_ps = stage0(blk)
                        live[blk] = [p_sb, y_ps, None]
                    if 1 <= blk <= N_BLKS:
                        live[blk - 1][2] = stage1(blk - 1, live[blk - 1][1])
                    if 2 <= blk:
                        p_sb_o, _, fe_o = live.pop(blk - 2)
                        stage2(blk - 2, p_sb_o, fe_o)

    nc.compile()
    return nc


def _prep_in_maps(u, W):
    x = u.reshape(N, CIN, H, W_SP)
    wl = _weight_tiles(W).reshape(128, 13 * 128)
    bd = _block_diag()
    ex = np.ascontiguousarray(bd.T)
    in_maps = []
    for core in range(N_CORES):
        n, half = core // 2, core % 2
        XA, XC = _inputs_core(x[n], half)
        in_maps.append({
            "xa": XA.reshape(128, XROWS * XCOLS),
            "xc": XC.reshape(128, XROWS * XCOLS),
            "wl": wl,
            "bd": bd,
            "ex": ex,
            "s2": _s2_map(half),
        })
    return in_maps


def run(u, W, trace=False):
    """Returns (out [N,T1,Z1,H,W] f32, BassKernelResults)."""
    from concourse.bass_utils import run_bass_kernel_spmd

    if "nc" not in _CACHE:
        _CACHE["nc"] = build_nc()
    nc = _CACHE["nc"]
    in_maps = _prep_in_maps(np.asarray(u, np.float32), np.asarray(W, np.float32))
    res = run_bass_kernel_spmd(nc, in_maps, list(range(N_CORES)), trace=trace)
    out = np.empty((N, T1, Z1, H, W_SP), np.float32)
    for core in range(N_CORES):
        n, half = core // 2, core % 2
        o = res.results[core]["out"].reshape(T1, Z1, ROWS, 128)
        out[n, :, :, half * 64:(half + 1) * 64, :] = o
    return out, res


def kernel(u, W):
    out, _ = run(u, W, trace=False)
    return out



# revision 3
# speedup vs baseline: 2.0860x; 2.0860x over previous
"""Trainium2 Bass kernel for nn_CapsuleLayer_9852654977072.

The reference module collapses mathematically: the routing loop's coupling
logits `b` stay zero (faithfully-reproduced bug in the original torch code),
so routing coefficients are a fixed spatial map r(h,w) = 1/(8*cnt(h,w)) where
cnt is the 5x5 box-count inside the image. The whole module is therefore:

    p = conv2d(u as [N,64,H,W], Wd as [128,64,5,5], pad=2) * s(h,w)
    v = squash_z1(p)   # groups of 16 channels
    out[n,t1,z1,h,w] = v

Device strategy (8 cores, SPMD): shard (batch n in 0..3) x (row-half in 0..1).
Each core computes all 128 output channels for 64 rows of one image.

Conv: inputs shipped as XA/XC [128, 68, 132] whose partition halves hold u
shifted by (+0row,+1row) and (+2row+0col,+2row+1col) respectively, columns
padded by 2. Per 4-row block, 13 PSUM-accumulated fp32r matmuls (N=512, full
PE rate) cover all 25 taps: 10 XA row-pairs + 2 XC col-pairs + 1 K=64 single.

Squash: per quarter (4 blocks), block-diagonal matmuls pack m2 = sum_z1 q^2
for all (block, t1) pairs into one [32, 512] PSUM tile (partition = 8*blk+t1).
The factor F = y/((1+y)*sqrt(y_raw+eps)), y = s^2*y_raw runs ONCE per quarter
on ACT/DVE (no GpSimd), then expand matmuls broadcast F back to the 128
channels and v = p * F. Expansion of quarter q-1 is emitted after the conv of
quarter q so the PE queue never stalls on the factor chain (keeps HAM warm).
"""

import numpy as np

T0, Z0, T1, Z1, KK, PAD = 4, 16, 8, 16, 5, 2
N, H, W_SP = 4, 128, 128
CIN, COUT = T0 * Z0, T1 * Z1  # 64, 128
N_CORES = 8
ROWS = 64          # output rows per core
XROWS = 68         # input rows incl. halo
XCOLS = 132        # 128 + 2*PAD
BLK = 4            # output rows per block
N_BLKS = ROWS // BLK   # 16
QBLKS = 4              # blocks per quarter
N_QTRS = N_BLKS // QBLKS

# conv matmul j -> (source, row_off, col_off); weights match in _weight_tiles
_MM_SLICES = (
    [('XA', dy + 2, dx + 2) for dy in (-2, 0) for dx in (-2, -1, 0, 1, 2)]
    + [('XC', 2, 0), ('XC', 2, 2), ('XC', 2, 4)]
)

_CACHE = {}


def _weight_tiles(W):
    Wd = W.transpose(1, 0, 2, 3, 4).reshape(COUT, CIN, KK, KK)
    wl = np.zeros((128, 13, 128), np.float32)  # [k, j, m]
    j = 0
    for dy in (-2, 0):
        for dx in (-2, -1, 0, 1, 2):
            wl[0:64, j, :] = Wd[:, :, dy + 2, dx + 2].T
            wl[64:128, j, :] = Wd[:, :, dy + 3, dx + 2].T
            j += 1
    for dx0 in (-2, 0):
        wl[0:64, j, :] = Wd[:, :, 4, dx0 + 2].T
        wl[64:128, j, :] = Wd[:, :, 4, dx0 + 3].T
        j += 1
    wl[0:64, j, :] = Wd[:, :, 4, 4].T  # single tap (2,2) on lo partitions
    return wl


def _inputs_core(x, half):
    """x: [64, H, W] one image channel-major. Returns XA, XC [128, 68, 132]."""
    base = half * 64 - 2
    XA = np.zeros((128, XROWS, XCOLS), np.float32)
    XC = np.zeros((128, XROWS, XCOLS), np.float32)

    def fill(dst, roff, c0, c1):
        lo, hi = max(0, -(base + roff)), min(XROWS, H - base - roff)
        dst[:, lo:hi, c0:c1] = x[:, base + roff + lo:base + roff + hi, :]

    fill(XA[0:64], 0, 2, 130)
    fill(XA[64:128], 1, 2, 130)
    fill(XC[0:64], 2, 2, 130)
    fill(XC[64:128], 2, 1, 129)
    return XA, XC


def _s2_quarters(half):
    """[32, N_QTRS, BLK*128] f32: s^2 at partition m=8*bp+t1 (t1-replicated),
    quarter q, flat pos = (row-within-block, col)."""
    idx = np.arange(H)
    cnt = (np.minimum(idx + 2, H - 1) - np.maximum(idx - 2, 0) + 1).astype(np.float64)
    s = 1.0 / (8.0 * cnt[:, None] * cnt[None, :])  # [H, W]
    s2 = (s * s)[half * 64:(half + 1) * 64, :]     # [64, 128]
    out = np.zeros((32, N_QTRS, BLK * 128), np.float64)
    for bp in range(QBLKS):
        for q in range(N_QTRS):
            blk = q * QBLKS + bp
            rows = s2[blk * BLK:(blk + 1) * BLK, :].reshape(-1)  # [512]
            out[8 * bp:8 * bp + 8, q, :] = rows[None, :]
    return np.ascontiguousarray(out.astype(np.float32).reshape(32, N_QTRS * BLK * 128))


def _bdq():
    """[128, QBLKS*32]: c=(t1,z1) -> partition m=8*bp+t1 summing over z1."""
    bd = np.zeros((128, QBLKS, 32), np.float32)
    c = np.arange(128)
    for bp in range(QBLKS):
        bd[c, bp, 8 * bp + c // 16] = 1.0
    return np.ascontiguousarray(bd.reshape(128, QBLKS * 32))


def _exq():
    """[32, QBLKS*128]: partition p=8*bp+t1 -> channels c with c//16==t1."""
    ex = np.zeros((32, QBLKS, 128), np.float32)
    c = np.arange(128)
    for bp in range(QBLKS):
        ex[8 * bp + c // 16, bp, c] = 1.0
    return np.ascontiguousarray(ex.reshape(32, QBLKS * 128))


def build_nc(reps=1):
    import concourse.bass as bass
    import concourse.bacc as bacc
    import concourse.mybir as mybir
    import concourse.tile as tile

    f32 = mybir.dt.float32
    f32r = mybir.dt.float32r
    AF = mybir.ActivationFunctionType

    nc = bacc.Bacc(None, target_bir_lowering=False)
    xa_d = nc.dram_tensor("xa", [128, XROWS * XCOLS], f32r, kind="ExternalInput")
    xc_d = nc.dram_tensor("xc", [128, XROWS * XCOLS], f32r, kind="ExternalInput")
    wl_d = nc.dram_tensor("wl", [128, 13 * 128], f32r, kind="ExternalInput")
    bdq_d = nc.dram_tensor("bdq", [128, QBLKS * 32], f32r, kind="ExternalInput")
    exq_d = nc.dram_tensor("exq", [32, QBLKS * 128], f32r, kind="ExternalInput")
    s2_d = nc.dram_tensor("s2", [32, N_QTRS * BLK * 128], f32, kind="ExternalInput")
    out_d = nc.dram_tensor("out", [128, ROWS * 128], f32, kind="ExternalOutput")

    with tile.TileContext(nc) as tc:
        with (
            tc.tile_pool(name="consts", bufs=1) as consts,
            tc.tile_pool(name="sq", bufs=3) as sq,
            tc.tile_pool(name="psb", bufs=9) as psb,
            tc.tile_pool(name="fac", bufs=2) as fac,
            tc.tile_pool(name="ff", bufs=2) as ff,
            tc.tile_pool(name="vv", bufs=4) as vv,
            tc.tile_pool(name="pp", bufs=3, space="PSUM") as pp,
            tc.tile_pool(name="py", bufs=2, space="PSUM") as py,
            tc.tile_pool(name="pf", bufs=2, space="PSUM") as pf,
        ):
            wl = consts.tile([128, 13, 128], f32r)
            nc.sync.dma_start(
                out=wl, in_=wl_d.ap().rearrange("p (j m) -> p j m", m=128))

            xa = consts.tile([128, XROWS, XCOLS], f32r)
            xc = consts.tile([128, XROWS, XCOLS], f32r)
            xa_src = xa_d.ap().rearrange("p (r c) -> p r c", c=XCOLS)
            xc_src = xc_d.ap().rearrange("p (r c) -> p r c", c=XCOLS)
            for c0 in range(0, XROWS, 17):
                nc.sync.dma_start(
                    out=xa[:, c0:c0 + 17, :], in_=xa_src[:, c0:c0 + 17, :])
                nc.sync.dma_start(
                    out=xc[:, c0:c0 + 17, :], in_=xc_src[:, c0:c0 + 17, :])

            bdq = consts.tile([128, QBLKS, 32], f32r)
            nc.sync.dma_start(
                out=bdq, in_=bdq_d.ap().rearrange("p (b m) -> p b m", m=32))
            exq = consts.tile([32, QBLKS, 128], f32r)
            nc.sync.dma_start(
                out=exq, in_=exq_d.ap().rearrange("p (b c) -> p b c", c=128))
            s2_sb = consts.tile([32, N_QTRS, BLK, 128], f32)
            nc.sync.dma_start(
                out=s2_sb,
                in_=s2_d.ap().rearrange("p (q r c) -> p q r c", r=BLK, c=128))
            eps_t = consts.tile([32, 1], f32)
            nc.vector.memset(eps_t[:], 1e-9)

            out_v = out_d.ap().rearrange("p (r c) -> p r c", c=128)

            import contextlib
            loop_ctx = (tc.For_i(0, reps, 1,
                                 hint_engines=(mybir.EngineType.PE,
                                               mybir.EngineType.DVE,
                                               mybir.EngineType.Activation,
                                               mybir.EngineType.SP))
                        if reps > 1 else contextlib.nullcontext())

            def conv_quarter(q, psbs):
                y_ps = py.tile([32, BLK, 128], f32)
                for bp in range(QBLKS):
                    blk = QBLKS * q + bp
                    r0 = blk * BLK
                    p_ps = pp.tile([128, BLK, 128], f32)
                    for j, (src, roff, coff) in enumerate(_MM_SLICES):
                        xsrc = xa if src == 'XA' else xc
                        if j == 12:  # K=64 single on lo partitions
                            lhsT = wl[0:64, j, :]
                            rhs = xsrc[0:64, r0 + roff:r0 + roff + BLK,
                                       coff:coff + 128]
                        else:
                            lhsT = wl[:, j, :]
                            rhs = xsrc[:, r0 + roff:r0 + roff + BLK,
                                       coff:coff + 128]
                        nc.tensor.matmul(p_ps[:], lhsT, rhs,
                                         start=(j == 0), stop=(j == 12))
                    psq = sq.tile([128, BLK, 128], f32r, tag="psq")
                    nc.scalar.activation(psq[:], p_ps[:], AF.Square)
                    p_sb = psb.tile([128, BLK, 128], f32, tag="psb")
                    nc.scalar.activation(p_sb[:], p_ps[:], AF.Copy, bias=0.0)
                    psbs[blk] = p_sb
                    nc.tensor.matmul(y_ps[:], bdq[:, bp, :], psq[:],
                                     start=(bp == 0), stop=(bp == QBLKS - 1))
                return y_ps

            def factor(q, y_ps):
                # F = y/((1+y)*sqrt(y_raw+eps)), y = s^2*y_raw, on [32, 512]
                a_t = fac.tile([32, BLK, 128], f32, tag="a")
                nc.scalar.activation(a_t[:], y_ps[:], AF.Sqrt, bias=eps_t[:])
                y_t = fac.tile([32, BLK, 128], f32, tag="y")
                nc.vector.tensor_mul(y_t[:], y_ps[:], s2_sb[:, q, :, :])
                y1_t = fac.tile([32, BLK, 128], f32, tag="y1")
                nc.scalar.activation(y1_t[:], y_t[:], AF.Copy, bias=1.0)
                b_t = fac.tile([32, BLK, 128], f32, tag="b")
                nc.vector.tensor_mul(b_t[:], a_t[:], y1_t[:])
                r_t = fac.tile([32, BLK, 128], f32, tag="r")
                nc.vector.reciprocal_approx_fast(r_t[:], b_t[:])
                F_t = ff.tile([32, BLK, 128], f32r, tag="F")
                nc.vector.tensor_mul(F_t[:], y_t[:], r_t[:])
                return F_t

            def expand(q, F_t, psbs):
                for bp in range(QBLKS):
                    blk = QBLKS * q + bp
                    r0 = blk * BLK
                    fe_ps = pf.tile([128, BLK, 128], f32)
                    nc.tensor.matmul(fe_ps[:], exq[:, bp, :], F_t[:],
                                     start=True, stop=True)
                    v_t = vv.tile([128, BLK, 128], f32, tag="v")
                    nc.vector.tensor_mul(v_t[:], psbs.pop(blk)[:], fe_ps[:])
                    nc.sync.dma_start(out=out_v[:, r0:r0 + BLK, :], in_=v_t[:])

            with loop_ctx:
                psbs, Fs = {}, {}
                for q in range(N_QTRS):
                    y_ps = conv_quarter(q, psbs)
                    if q >= 1:
                        expand(q - 1, Fs.pop(q - 1), psbs)
                    Fs[q] = factor(q, y_ps)
                expand(N_QTRS - 1, Fs.pop(N_QTRS - 1), psbs)

    nc.compile()
    return nc


def _prep_in_maps(u, W):
    x = u.reshape(N, CIN, H, W_SP)
    wl = _weight_tiles(W).reshape(128, 13 * 128)
    bdq = _bdq()
    exq = _exq()
    s2q = [_s2_quarters(half) for half in range(2)]
    in_maps = []
    for core in range(N_CORES):
        n, half = core // 2, core % 2
        XA, XC = _inputs_core(x[n], half)
        in_maps.append({
            "xa": XA.reshape(128, XROWS * XCOLS),
            "xc": XC.reshape(128, XROWS * XCOLS),
            "wl": wl,
            "bdq": bdq,
            "exq": exq,
            "s2": s2q[half],
        })
    return in_maps


def run(u, W, trace=False):
    """Returns (out [N,T1,Z1,H,W] f32, BassKernelResults)."""
    from concourse.bass_utils import run_bass_kernel_spmd

    if "nc" not in _CACHE:
        _CACHE["nc"] = build_nc()
    nc = _CACHE["nc"]
    in_maps = _prep_in_maps(np.asarray(u, np.float32), np.asarray(W, np.float32))
    res = run_bass_kernel_spmd(nc, in_maps, list(range(N_CORES)), trace=trace)
    out = np.empty((N, T1, Z1, H, W_SP), np.float32)
    for core in range(N_CORES):
        n, half = core // 2, core % 2
        o = res.results[core]["out"].reshape(T1, Z1, ROWS, 128)
        out[n, :, :, half * 64:(half + 1) * 64, :] = o
    return out, res


def kernel(u, W):
    out, _ = run(u, W, trace=False)
    return out


# revision 5
# speedup vs baseline: 2.1410x; 1.0264x over previous
"""Trainium2 Bass kernel for nn_CapsuleLayer_9852654977072.

The reference module collapses mathematically: the routing loop's coupling
logits `b` stay zero (faithfully-reproduced bug in the original torch code),
so routing coefficients are a fixed spatial map r(h,w) = 1/(8*cnt(h,w)) where
cnt is the 5x5 box-count inside the image. The whole module is therefore:

    p = conv2d(u as [N,64,H,W], Wd as [128,64,5,5], pad=2) * s(h,w)
    v = squash_z1(p)   # groups of 16 channels
    out[n,t1,z1,h,w] = v

Device strategy (8 cores, SPMD): shard (batch n in 0..3) x (row-half in 0..1).
Each core computes all 128 output channels for 64 rows of one image.

Conv: inputs shipped as XA/XC [128, 68, 132] whose partition halves hold u
shifted by (+0row,+1row) and (+2row+0col,+2row+1col) respectively, columns
padded by 2. Per 4-row block, 13 PSUM-accumulated fp32r matmuls (N=512, full
PE rate) cover all 25 taps: 10 XA row-pairs + 2 XC col-pairs + 1 K=64 single.

Squash: per quarter (4 blocks), block-diagonal matmuls pack m2 = sum_z1 q^2
for all (block, t1) pairs into one [32, 512] PSUM tile (partition = 8*blk+t1).
The factor F = y/((1+y)*sqrt(y_raw+eps)), y = s^2*y_raw runs ONCE per quarter
on ACT/DVE (no GpSimd), then expand matmuls broadcast F back to the 128
channels and v = p * F. Expansion of quarter q-1 is emitted after the conv of
quarter q so the PE queue never stalls on the factor chain (keeps HAM warm).
"""

import numpy as np

T0, Z0, T1, Z1, KK, PAD = 4, 16, 8, 16, 5, 2
N, H, W_SP = 4, 128, 128
CIN, COUT = T0 * Z0, T1 * Z1  # 64, 128
N_CORES = 8
ROWS = 64          # output rows per core
XROWS = 68         # input rows incl. halo
XCOLS = 132        # 128 + 2*PAD
BLK = 4            # output rows per block
N_BLKS = ROWS // BLK   # 16
QBLKS = 4              # blocks per quarter
N_QTRS = N_BLKS // QBLKS

# conv matmul j -> (source, row_off, col_off); weights match in _weight_tiles
_MM_SLICES = (
    [('XA', dy + 2, dx + 2) for dy in (-2, 0) for dx in (-2, -1, 0, 1, 2)]
    + [('XC', 2, 0), ('XC', 2, 2), ('XC', 2, 4)]
)

_CACHE = {}


def _weight_tiles(W):
    Wd = W.transpose(1, 0, 2, 3, 4).reshape(COUT, CIN, KK, KK)
    wl = np.zeros((128, 13, 128), np.float32)  # [k, j, m]
    j = 0
    for dy in (-2, 0):
        for dx in (-2, -1, 0, 1, 2):
            wl[0:64, j, :] = Wd[:, :, dy + 2, dx + 2].T
            wl[64:128, j, :] = Wd[:, :, dy + 3, dx + 2].T
            j += 1
    for dx0 in (-2, 0):
        wl[0:64, j, :] = Wd[:, :, 4, dx0 + 2].T
        wl[64:128, j, :] = Wd[:, :, 4, dx0 + 3].T
        j += 1
    wl[0:64, j, :] = Wd[:, :, 4, 4].T  # single tap (2,2) on lo partitions
    return wl


def _inputs_core(x, half):
    """x: [64, H, W] one image channel-major. Returns XA, XC [128, 68, 132]."""
    base = half * 64 - 2
    XA = np.zeros((128, XROWS, XCOLS), np.float32)
    XC = np.zeros((128, XROWS, XCOLS), np.float32)

    def fill(dst, roff, c0, c1):
        lo, hi = max(0, -(base + roff)), min(XROWS, H - base - roff)
        dst[:, lo:hi, c0:c1] = x[:, base + roff + lo:base + roff + hi, :]

    fill(XA[0:64], 0, 2, 130)
    fill(XA[64:128], 1, 2, 130)
    fill(XC[0:64], 2, 2, 130)
    fill(XC[64:128], 2, 1, 129)
    return XA, XC


def _s2_quarters(half):
    """[32, N_QTRS, BLK*128] f32: s^2 at partition m=8*bp+t1 (t1-replicated),
    quarter q, flat pos = (row-within-block, col)."""
    idx = np.arange(H)
    cnt = (np.minimum(idx + 2, H - 1) - np.maximum(idx - 2, 0) + 1).astype(np.float64)
    s = 1.0 / (8.0 * cnt[:, None] * cnt[None, :])  # [H, W]
    s2 = (s * s)[half * 64:(half + 1) * 64, :]     # [64, 128]
    out = np.zeros((32, N_QTRS, BLK * 128), np.float64)
    for bp in range(QBLKS):
        for q in range(N_QTRS):
            blk = q * QBLKS + bp
            rows = s2[blk * BLK:(blk + 1) * BLK, :].reshape(-1)  # [512]
            out[8 * bp:8 * bp + 8, q, :] = rows[None, :]
    return np.ascontiguousarray(out.astype(np.float32).reshape(32, N_QTRS * BLK * 128))


def _bdq():
    """[128, QBLKS*32]: c=(t1,z1) -> partition m=8*bp+t1 summing over z1."""
    bd = np.zeros((128, QBLKS, 32), np.float32)
    c = np.arange(128)
    for bp in range(QBLKS):
        bd[c, bp, 8 * bp + c // 16] = 1.0
    return np.ascontiguousarray(bd.reshape(128, QBLKS * 32))


def _exq():
    """[32, QBLKS*128]: partition p=8*bp+t1 -> channels c with c//16==t1."""
    ex = np.zeros((32, QBLKS, 128), np.float32)
    c = np.arange(128)
    for bp in range(QBLKS):
        ex[8 * bp + c // 16, bp, c] = 1.0
    return np.ascontiguousarray(ex.reshape(32, QBLKS * 128))


def build_nc(reps=1):
    import concourse.bass as bass
    import concourse.bacc as bacc
    import concourse.mybir as mybir
    import concourse.tile as tile

    f32 = mybir.dt.float32
    f32r = mybir.dt.float32r
    AF = mybir.ActivationFunctionType

    nc = bacc.Bacc(None, target_bir_lowering=False)
    xa_d = nc.dram_tensor("xa", [128, XROWS * XCOLS], f32r, kind="ExternalInput")
    xc_d = nc.dram_tensor("xc", [128, XROWS * XCOLS], f32r, kind="ExternalInput")
    wl_d = nc.dram_tensor("wl", [128, 13 * 128], f32r, kind="ExternalInput")
    bdq_d = nc.dram_tensor("bdq", [128, QBLKS * 32], f32r, kind="ExternalInput")
    exq_d = nc.dram_tensor("exq", [32, QBLKS * 128], f32r, kind="ExternalInput")
    s2_d = nc.dram_tensor("s2", [32, N_QTRS * BLK * 128], f32, kind="ExternalInput")
    out_d = nc.dram_tensor("out", [128, ROWS * 128], f32, kind="ExternalOutput")

    with tile.TileContext(nc) as tc:
        with (
            tc.tile_pool(name="consts", bufs=1) as consts,
            tc.tile_pool(name="sq", bufs=3) as sq,
            tc.tile_pool(name="psb", bufs=9) as psb,
            tc.tile_pool(name="fac", bufs=2) as fac,
            tc.tile_pool(name="ff", bufs=2) as ff,
            tc.tile_pool(name="vv", bufs=4) as vv,
            tc.tile_pool(name="pp", bufs=3, space="PSUM") as pp,
            tc.tile_pool(name="py", bufs=2, space="PSUM") as py,
            tc.tile_pool(name="pf", bufs=2, space="PSUM") as pf,
            tc.tile_pool(name="pd", bufs=1, space="PSUM") as pd,
        ):
            # PE pre-warm: dummy matmuls on a zeroed tile keep the PE busy
            # during the input-DMA lead-in so HAM un-throttles to 2.4 GHz
            # before the first real conv matmul.
            dum = consts.tile([128, 512], f32)
            nc.vector.memset(dum[:], 0.0)
            dum_ps = pd.tile([128, 512], f32)
            for _ in range(12):
                nc.tensor.matmul(dum_ps[:], dum[:, 0:128].bitcast(f32r),
                                 dum[:].bitcast(f32r), start=True, stop=True)

            wl = consts.tile([128, 13, 128], f32r)
            wl_src = wl_d.ap().rearrange("p (j m) -> p j m", m=128)
            nc.sync.dma_start(out=wl[:, 0:2, :], in_=wl_src[:, 0:2, :])

            xa = consts.tile([128, XROWS, XCOLS], f32r)
            xc = consts.tile([128, XROWS, XCOLS], f32r)
            xa_src = xa_d.ap().rearrange("p (r c) -> p r c", c=XCOLS)
            xc_src = xc_d.ap().rearrange("p (r c) -> p r c", c=XCOLS)
            # First chunks cover only block 0 (xa rows 0-5, xc rows 2-5) to
            # minimize the time to the first conv matmul; xc rows 0-1/66-67
            # are never read.
            xa_chunks = [(0, 6), (6, 23), (23, 40), (40, 57), (57, 68)]
            xc_chunks = [(2, 6), (6, 23), (23, 40), (40, 57), (57, 66)]
            nc.sync.dma_start(out=xa[:, 0:6, :], in_=xa_src[:, 0:6, :])
            nc.sync.dma_start(out=xc[:, 2:6, :], in_=xc_src[:, 2:6, :])
            nc.sync.dma_start(out=wl[:, 2:13, :], in_=wl_src[:, 2:13, :])
            for (a0, a1), (c0, c1) in zip(xa_chunks[1:], xc_chunks[1:]):
                nc.sync.dma_start(out=xa[:, a0:a1, :], in_=xa_src[:, a0:a1, :])
                nc.sync.dma_start(out=xc[:, c0:c1, :], in_=xc_src[:, c0:c1, :])

            bdq = consts.tile([128, QBLKS, 32], f32r)
            nc.sync.dma_start(
                out=bdq, in_=bdq_d.ap().rearrange("p (b m) -> p b m", m=32))
            exq = consts.tile([32, QBLKS, 128], f32r)
            nc.sync.dma_start(
                out=exq, in_=exq_d.ap().rearrange("p (b c) -> p b c", c=128))
            s2_sb = consts.tile([32, N_QTRS, BLK, 128], f32)
            nc.sync.dma_start(
                out=s2_sb,
                in_=s2_d.ap().rearrange("p (q r c) -> p q r c", r=BLK, c=128))
            eps_t = consts.tile([32, 1], f32)
            nc.vector.memset(eps_t[:], 1e-9)

            out_v = out_d.ap().rearrange("p (r c) -> p r c", c=128)

            import contextlib
            loop_ctx = (tc.For_i(0, reps, 1,
                                 hint_engines=(mybir.EngineType.PE,
                                               mybir.EngineType.DVE,
                                               mybir.EngineType.Activation,
                                               mybir.EngineType.SP))
                        if reps > 1 else contextlib.nullcontext())

            def conv_quarter(q, psbs):
                y_ps = py.tile([32, BLK, 128], f32)
                for bp in range(QBLKS):
                    blk = QBLKS * q + bp
                    r0 = blk * BLK
                    p_ps = pp.tile([128, BLK, 128], f32)
                    for j, (src, roff, coff) in enumerate(_MM_SLICES):
                        xsrc = xa if src == 'XA' else xc
                        if j == 12:  # K=64 single on lo partitions
                            lhsT = wl[0:64, j, :]
                            rhs = xsrc[0:64, r0 + roff:r0 + roff + BLK,
                                       coff:coff + 128]
                        else:
                            lhsT = wl[:, j, :]
                            rhs = xsrc[:, r0 + roff:r0 + roff + BLK,
                                       coff:coff + 128]
                        nc.tensor.matmul(p_ps[:], lhsT, rhs,
                                         start=(j == 0), stop=(j == 12))
                    psq = sq.tile([128, BLK, 128], f32r, tag="psq")
                    nc.scalar.activation(psq[:], p_ps[:], AF.Square)
                    p_sb = psb.tile([128, BLK, 128], f32, tag="psb")
                    nc.scalar.activation(p_sb[:], p_ps[:], AF.Copy, bias=0.0)
                    psbs[blk] = p_sb
                    nc.tensor.matmul(y_ps[:], bdq[:, bp, :], psq[:],
                                     start=(bp == 0), stop=(bp == QBLKS - 1))
                return y_ps

            def factor(q, y_ps):
                # F = y/((1+y)*sqrt(y_raw+eps)), y = s^2*y_raw, on [32, 512]
                a_t = fac.tile([32, BLK, 128], f32, tag="a")
                nc.scalar.activation(a_t[:], y_ps[:], AF.Sqrt, bias=eps_t[:])
                y_t = fac.tile([32, BLK, 128], f32, tag="y")
                nc.vector.tensor_mul(y_t[:], y_ps[:], s2_sb[:, q, :, :])
                y1_t = fac.tile([32, BLK, 128], f32, tag="y1")
                nc.scalar.activation(y1_t[:], y_t[:], AF.Copy, bias=1.0)
                b_t = fac.tile([32, BLK, 128], f32, tag="b")
                nc.vector.tensor_mul(b_t[:], a_t[:], y1_t[:])
                r_t = fac.tile([32, BLK, 128], f32, tag="r")
                nc.vector.reciprocal_approx_fast(r_t[:], b_t[:])
                F_t = ff.tile([32, BLK, 128], f32r, tag="F")
                nc.vector.tensor_mul(F_t[:], y_t[:], r_t[:])
                return F_t

            def expand(q, F_t, psbs):
                for bp in range(QBLKS):
                    blk = QBLKS * q + bp
                    r0 = blk * BLK
                    fe_ps = pf.tile([128, BLK, 128], f32)
                    nc.tensor.matmul(fe_ps[:], exq[:, bp, :], F_t[:],
                                     start=True, stop=True)
                    v_t = vv.tile([128, BLK, 128], f32, tag="v")
                    nc.vector.tensor_mul(v_t[:], psbs.pop(blk)[:], fe_ps[:])
                    nc.sync.dma_start(out=out_v[:, r0:r0 + BLK, :], in_=v_t[:])

            with loop_ctx:
                psbs, Fs = {}, {}
                for q in range(N_QTRS):
                    y_ps = conv_quarter(q, psbs)
                    if q >= 1:
                        expand(q - 1, Fs.pop(q - 1), psbs)
                    Fs[q] = factor(q, y_ps)
                expand(N_QTRS - 1, Fs.pop(N_QTRS - 1), psbs)

    nc.compile()
    return nc


def _prep_in_maps(u, W):
    x = u.reshape(N, CIN, H, W_SP)
    wl = _weight_tiles(W).reshape(128, 13 * 128)
    bdq = _bdq()
    exq = _exq()
    s2q = [_s2_quarters(half) for half in range(2)]
    in_maps = []
    for core in range(N_CORES):
        n, half = core // 2, core % 2
        XA, XC = _inputs_core(x[n], half)
        in_maps.append({
            "xa": XA.reshape(128, XROWS * XCOLS),
            "xc": XC.reshape(128, XROWS * XCOLS),
            "wl": wl,
            "bdq": bdq,
            "exq": exq,
            "s2": s2q[half],
        })
    return in_maps


def run(u, W, trace=False):
    """Returns (out [N,T1,Z1,H,W] f32, BassKernelResults)."""
    from concourse.bass_utils import run_bass_kernel_spmd

    if "nc" not in _CACHE:
        _CACHE["nc"] = build_nc()
    nc = _CACHE["nc"]
    in_maps = _prep_in_maps(np.asarray(u, np.float32), np.asarray(W, np.float32))
    res = run_bass_kernel_spmd(nc, in_maps, list(range(N_CORES)), trace=trace)
    out = np.empty((N, T1, Z1, H, W_SP), np.float32)
    for core in range(N_CORES):
        n, half = core // 2, core % 2
        o = res.results[core]["out"].reshape(T1, Z1, ROWS, 128)
        out[n, :, :, half * 64:(half + 1) * 64, :] = o
    return out, res


def kernel(u, W):
    out, _ = run(u, W, trace=False)
    return out
